# revision 1
# baseline (speedup 1.0000x reference)
"""Additive-attention scoring kernel for Trainium2 (Bass/Tile, 8 NeuronCores).

Computes softmax_t( v . tanh( W @ cat(hidden, enc)[b,t] + b ) ) for
hidden (B,H), enc (B,T,2H), W (H,3H), b (H,), v (H,)  ->  (B,1,T).

Restructured vs the classic layout: enc chunks are the STATIONARY operand
([k,t] 128x128) and W2 streams as the MOVING operand ([k,h] 128x512), so
PSUM tiles are [t, h].  Benefits:
  - the v-dot reduction is over the FREE axis -> fused mult+reduce on the
    (otherwise idle) Vector engine instead of 8 extra PE matmuls/t-tile
  - each stationary serves 2 moving matmuls (h halves) -> half the
    LDWEIGHTS traffic
  - scores land as [128t, 16] across partitions -> softmax tail is
    parallel instead of serial on one partition
  - u[b] = W1 @ hidden[b] + b is computed on the HOST (tiny GEMM) and
    shipped replicated across partitions; no W1/hidden on device

Sharding: data-parallel over batch, 2 batches per core.
"""

import numpy as np
import ml_dtypes

B, T, H = 16, 2048, 1024
K2 = 2 * H          # contraction dim of the big matmul
NCORES = 8
BPC = B // NCORES   # batches per core

P = 128
KO = K2 // P        # 16 k-chunks
TT = 512            # t-tile (et DMA granularity)
NTT = T // TT       # 4 et tiles per batch
TQ = TT // P        # 4 t-chunks per et tile
NTC = T // P        # 16 t-chunks per batch
HH = H // 2         # moving h-half (one PSUM bank of fp32)

_BF16 = ml_dtypes.bfloat16

_nc_cache = None
_in_maps_cache = None


def _build_nc(repeat=1):
    """Build the SPMD kernel. repeat>1 duplicates the compute body inside the
    NEFF (same inputs/outputs) — used only for differential device timing."""
    from contextlib import ExitStack

    import concourse.tile as tile
    from concourse import bacc, mybir

    f32 = mybir.dt.float32
    bf16 = mybir.dt.bfloat16
    AF = mybir.ActivationFunctionType
    ALU = mybir.AluOpType

    nc = bacc.Bacc()

    enct = nc.dram_tensor("enct", [BPC, K2, T], bf16, kind="ExternalInput")
    w2t = nc.dram_tensor("w2t", [K2, H], bf16, kind="ExternalInput")
    ubc = nc.dram_tensor("ubc", [P, BPC, H], f32, kind="ExternalInput")
    vbc = nc.dram_tensor("vbc", [P, H], bf16, kind="ExternalInput")
    ones = nc.dram_tensor("ones", [P, P], bf16, kind="ExternalInput")
    out = nc.dram_tensor("out", [BPC, NTC, P], f32, kind="ExternalOutput")

    with tile.TileContext(nc) as tc, ExitStack() as ctx:
        consts = ctx.enter_context(tc.tile_pool(name="consts", bufs=1))
        enc_pool = ctx.enter_context(tc.tile_pool(name="enc", bufs=4))
        th_pool = ctx.enter_context(tc.tile_pool(name="th", bufs=3))
        eu_pool = ctx.enter_context(tc.tile_pool(name="eu", bufs=2))
        vth_pool = ctx.enter_context(tc.tile_pool(name="vth", bufs=2))
        pe_pool = ctx.enter_context(tc.tile_pool(name="pe", bufs=6, space="PSUM"))
        ps_pool = ctx.enter_context(tc.tile_pool(name="ps", bufs=1, space="PSUM"))
        small = ctx.enter_context(tc.tile_pool(name="small", bufs=4))

        # Loads chunked in consumption order: the first matmuls need only
        # w2[kc=0] + et0[kc=0]; per-kc DMA (384 KB ~ 1.1 us) stays ahead of
        # the PE's 6 matmuls/kc (~1.3 us) in the first tile's kc-outer phase.
        w2_sb = consts.tile([P, KO, H], bf16)
        w2_r = w2t.rearrange("(ko p) h -> p ko h", p=P)
        ub_sb = consts.tile([P, BPC, H], f32)
        vb_sb = consts.tile([P, H], bf16)
        ones_sb = consts.tile([P, P], bf16)
        enct_b0 = enct[0].rearrange("(ko p) t -> p ko t", p=P)
        et0 = enc_pool.tile([P, KO, TT], bf16)
        for g in range(KO // 2):
            ko = 2 * g
            nc.sync.dma_start(w2_sb[:, ko : ko + 2, :], w2_r[:, ko : ko + 2, :])
            eng = nc.gpsimd if g < 2 else nc.sync
            eng.dma_start(et0[:, ko : ko + 2, :], enct_b0[:, ko : ko + 2, 0:TT])
            if g == 1:
                nc.gpsimd.dma_start(ub_sb, ubc[:])
                nc.gpsimd.dma_start(vb_sb, vbc[:])
                nc.gpsimd.dma_start(ones_sb, ones[:])

        def tc_epilogue(bi, tcg, peL, peR, score):
            # DVE adds u (reading PSUM, writing SBUF — DVE never writes
            # PSUM, PE-drain vs engine-access hazards stay one-directional),
            # ACT applies tanh, then one fused DVE mult+reduce forms the
            # t-chunk scores.
            eu = eu_pool.tile([P, 2 * TT], f32, tag="eu")
            th = th_pool.tile([P, 2 * TT], bf16, tag="th")
            nc.vector.tensor_add(eu[:, 0:TT], peL, ub_sb[:, bi, 0:HH])
            nc.scalar.activation(th[:, 0:TT], eu[:, 0:TT], AF.Tanh)
            nc.vector.tensor_add(eu[:, TT : 2 * TT], peR, ub_sb[:, bi, HH:H])
            nc.scalar.activation(th[:, TT : 2 * TT], eu[:, TT : 2 * TT], AF.Tanh)
            vth = vth_pool.tile([P, 2 * TT], bf16, tag="vth")
            nc.vector.tensor_mul(vth, th, vb_sb)
            nc.vector.tensor_reduce(
                score[:, tcg : tcg + 1], vth, axis=mybir.AxisListType.X,
                op=ALU.add,
            )

        first = True
        for bi in [bi for _ in range(repeat) for bi in range(BPC)]:
            enct_b = enct[bi].rearrange("(ko p) t -> p ko t", p=P)
            score = small.tile([P, NTC], f32, tag="score")
            for ti in range(NTT):
                if first:
                    et = et0
                else:
                    et = enc_pool.tile([P, KO, TT], bf16, tag="et0")
                    for ko in range(0, KO, 4):
                        nc.sync.dma_start(
                            et[:, ko : ko + 4, :],
                            enct_b[:, ko : ko + 4, ti * TT : (ti + 1) * TT],
                        )
                if first:
                    # kc-outer over t-chunks 0-2 so the PE starts as soon as
                    # the first w2/et chunks land; tc 3 runs kc-inner after.
                    first = False
                    pes = [
                        (
                            pe_pool.tile([P, TT], f32, tag="pe", name=f"peL{i}"),
                            pe_pool.tile([P, TT], f32, tag="pe", name=f"peR{i}"),
                        )
                        for i in range(3)
                    ]
                    for kc in range(KO):
                        for tq in range(3):
                            st = et[:, kc, tq * P : (tq + 1) * P]
                            peL, peR = pes[tq]
                            nc.tensor.matmul(
                                peL, st, w2_sb[:, kc, 0:HH],
                                start=(kc == 0), stop=(kc == KO - 1),
                            )
                            nc.tensor.matmul(
                                peR, st, w2_sb[:, kc, HH:H],
                                start=(kc == 0), stop=(kc == KO - 1),
                            )
                    for tq in range(3):
                        tc_epilogue(bi, tq, pes[tq][0], pes[tq][1], score)
                    tqs = [3]
                else:
                    tqs = range(TQ)
                for tq in tqs:
                    # serial accumulation groups: consecutive matmuls to the
                    # same PSUM bank sustain ~272 ns/mm; alternating banks
                    # per kc (to share the stationary) measures ~304 ns/mm.
                    peL = pe_pool.tile([P, TT], f32, tag="pe")
                    peR = pe_pool.tile([P, TT], f32, tag="pe")
                    for kc in range(KO):
                        nc.tensor.matmul(
                            peL, et[:, kc, tq * P : (tq + 1) * P],
                            w2_sb[:, kc, 0:HH],
                            start=(kc == 0), stop=(kc == KO - 1),
                        )
                    for kc in range(KO):
                        nc.tensor.matmul(
                            peR, et[:, kc, tq * P : (tq + 1) * P],
                            w2_sb[:, kc, HH:H],
                            start=(kc == 0), stop=(kc == KO - 1),
                        )
                    tc_epilogue(bi, ti * TQ + tq, peL, peR, score)
            # batch tail: exp over [128, 16] with free-axis partial sums,
            # partition-sum via a tiny fp32 ones-matmul, then normalize.
            probs = small.tile([P, NTC], f32, tag="probs")
            ssum = small.tile([P, 1], f32, tag="ssum")
            nc.scalar.activation(probs, score, AF.Exp, accum_out=ssum)
            # partition-sum via a bf16 ones-matmul; hi/lo split keeps the
            # normalizer at ~fp24 (a single bf16 cast would cost 0.4% rel)
            ssb = small.tile([P, 2], bf16, tag="ssb")
            nc.vector.tensor_copy(ssb[:, 0:1], ssum)
            nc.vector.tensor_sub(ssb[:, 1:2], ssum, ssb[:, 0:1])
            psS = ps_pool.tile([P, 2], f32, tag="psS")
            nc.tensor.matmul(psS, ones_sb, ssb, start=True, stop=True)
            rs = small.tile([P, 1], f32, tag="rs")
            sS = small.tile([P, 1], f32, tag="sS")
            nc.vector.tensor_reduce(
                sS, psS, axis=mybir.AxisListType.X, op=ALU.add
            )
            nc.vector.reciprocal(rs, sS)
            probn = small.tile([P, NTC], f32, tag="probn")
            nc.vector.tensor_scalar_mul(probn, probs, rs)
            nc.sync.dma_start(out[bi].rearrange("tc p -> p tc"), probn)

    nc.compile()
    return nc


def kernel(hidden, encoder_outputs, W, b, v):
    global _nc_cache, _in_maps_cache
    from concourse.bass_utils import run_bass_kernel_spmd

    hidden = np.asarray(hidden, dtype=np.float32)
    enc = np.asarray(encoder_outputs, dtype=np.float32)
    W = np.asarray(W, dtype=np.float32)
    b = np.asarray(b, dtype=np.float32)
    v = np.asarray(v, dtype=np.float32)

    w2t = np.ascontiguousarray(W[:, H:].T).astype(_BF16)      # (2H, H) bf16
    u = hidden @ W[:, :H].T + b                               # (B, H) fp32
    vb = np.ascontiguousarray(
        np.broadcast_to(v.astype(_BF16)[None], (P, H))
    )
    ones = np.ones((P, P), dtype=_BF16)
    # (B, 2H, T) bf16 — contraction dim on partitions, t contiguous
    enct = np.ascontiguousarray(enc.transpose(0, 2, 1)).astype(_BF16)

    if _nc_cache is None:
        _nc_cache = _build_nc()
    nc = _nc_cache

    in_maps = []
    for c in range(NCORES):
        bs = c * BPC
        in_maps.append(
            {
                "enct": enct[bs : bs + BPC],
                "w2t": w2t,
                "ubc": np.ascontiguousarray(
                    np.broadcast_to(u[None, bs : bs + BPC], (P, BPC, H))
                ),
                "vbc": vb,
                "ones": ones,
            }
        )

    _in_maps_cache = in_maps
    res = run_bass_kernel_spmd(nc, in_maps, list(range(NCORES)))
    # device layout is [BPC, NTC, P]; t = tc*P + p
    outs = [res.results[c]["out"].reshape(BPC, T) for c in range(NCORES)]
    return np.concatenate(outs, axis=0)[:, None, :].astype(np.float32)



# revision 2
# speedup vs baseline: 1.0258x; 1.0258x over previous
"""Additive-attention scoring kernel for Trainium2 (Bass/Tile, 8 NeuronCores).

Computes softmax_t( v . tanh( W @ cat(hidden, enc)[b,t] + b ) ) for
hidden (B,H), enc (B,T,2H), W (H,3H), b (H,), v (H,)  ->  (B,1,T).

Restructured vs the classic layout: enc chunks are the STATIONARY operand
([k,t] 128x128) and W2 streams as the MOVING operand ([k,h] 128x512), so
PSUM tiles are [t, h].  Benefits:
  - the v-dot reduction is over the FREE axis -> fused mult+reduce on the
    (otherwise idle) Vector engine instead of 8 extra PE matmuls/t-tile
  - each stationary serves 2 moving matmuls (h halves) -> half the
    LDWEIGHTS traffic
  - scores land as [128t, 16] across partitions -> softmax tail is
    parallel instead of serial on one partition
  - u[b] = W1 @ hidden[b] + b is computed on the HOST (tiny GEMM) and
    shipped replicated across partitions; no W1/hidden on device

Sharding: data-parallel over batch, 2 batches per core.
"""

import numpy as np
import ml_dtypes

B, T, H = 16, 2048, 1024
K2 = 2 * H          # contraction dim of the big matmul
NCORES = 8
BPC = B // NCORES   # batches per core

P = 128
KO = K2 // P        # 16 k-chunks
TT = 512            # t-tile (et DMA granularity)
NTT = T // TT       # 4 et tiles per batch
TQ = TT // P        # 4 t-chunks per et tile
NTC = T // P        # 16 t-chunks per batch
HH = H // 2         # moving h-half (one PSUM bank of fp32)

_BF16 = ml_dtypes.bfloat16

_nc_cache = None
_in_maps_cache = None


def _build_nc(repeat=1):
    """Build the SPMD kernel. repeat>1 duplicates the compute body inside the
    NEFF (same inputs/outputs) — used only for differential device timing."""
    from contextlib import ExitStack

    import concourse.tile as tile
    from concourse import bacc, mybir

    f32 = mybir.dt.float32
    bf16 = mybir.dt.bfloat16
    AF = mybir.ActivationFunctionType
    ALU = mybir.AluOpType

    nc = bacc.Bacc()

    enct = nc.dram_tensor("enct", [BPC, K2, T], bf16, kind="ExternalInput")
    w2t = nc.dram_tensor("w2t", [K2, H], bf16, kind="ExternalInput")
    ubc = nc.dram_tensor("ubc", [P, BPC, H], f32, kind="ExternalInput")
    vbc = nc.dram_tensor("vbc", [P, H], bf16, kind="ExternalInput")
    ones = nc.dram_tensor("ones", [P, P], bf16, kind="ExternalInput")
    out = nc.dram_tensor("out", [BPC, NTC, P], f32, kind="ExternalOutput")

    with tile.TileContext(nc) as tc, ExitStack() as ctx:
        consts = ctx.enter_context(tc.tile_pool(name="consts", bufs=1))
        enc_pool = ctx.enter_context(tc.tile_pool(name="enc", bufs=4))
        th_pool = ctx.enter_context(tc.tile_pool(name="th", bufs=3))
        eu_pool = ctx.enter_context(tc.tile_pool(name="eu", bufs=2))
        vth_pool = ctx.enter_context(tc.tile_pool(name="vth", bufs=2))
        pe_pool = ctx.enter_context(tc.tile_pool(name="pe", bufs=6, space="PSUM"))
        ps_pool = ctx.enter_context(tc.tile_pool(name="ps", bufs=1, space="PSUM"))
        small = ctx.enter_context(tc.tile_pool(name="small", bufs=4))

        # Loads chunked in consumption order: the first matmuls need only
        # w2[kc=0] + et0[kc=0]; per-kc DMA (384 KB ~ 1.1 us) stays ahead of
        # the PE's 6 matmuls/kc (~1.3 us) in the first tile's kc-outer phase.
        w2_sb = consts.tile([P, KO, H], bf16)
        w2_r = w2t.rearrange("(ko p) h -> p ko h", p=P)
        ub_sb = consts.tile([P, BPC, H], f32)
        vb_sb = consts.tile([P, H], bf16)
        ones_sb = consts.tile([P, P], bf16)
        enct_b0 = enct[0].rearrange("(ko p) t -> p ko t", p=P)
        et0 = enc_pool.tile([P, KO, TT], bf16)
        for g in range(KO // 2):
            ko = 2 * g
            nc.sync.dma_start(w2_sb[:, ko : ko + 2, :], w2_r[:, ko : ko + 2, :])
            eng = nc.gpsimd if g < 2 else nc.sync
            eng.dma_start(et0[:, ko : ko + 2, :], enct_b0[:, ko : ko + 2, 0:TT])
            if g == 1:
                nc.gpsimd.dma_start(ub_sb, ubc[:])
                nc.gpsimd.dma_start(vb_sb, vbc[:])
                nc.gpsimd.dma_start(ones_sb, ones[:])

        def tc_epilogue(bi, tcg, peL, peR, score):
            # DVE adds u (reading PSUM, writing SBUF — DVE never writes
            # PSUM, PE-drain vs engine-access hazards stay one-directional),
            # ACT applies tanh, then one fused DVE mult+reduce forms the
            # t-chunk scores.
            eu = eu_pool.tile([P, 2 * TT], f32, tag="eu")
            th = th_pool.tile([P, 2 * TT], bf16, tag="th")
            nc.vector.tensor_add(eu[:, 0:TT], peL, ub_sb[:, bi, 0:HH])
            nc.scalar.activation(th[:, 0:TT], eu[:, 0:TT], AF.Tanh)
            nc.vector.tensor_add(eu[:, TT : 2 * TT], peR, ub_sb[:, bi, HH:H])
            nc.scalar.activation(th[:, TT : 2 * TT], eu[:, TT : 2 * TT], AF.Tanh)
            vth = vth_pool.tile([P, 2 * TT], bf16, tag="vth")
            nc.vector.tensor_mul(vth, th, vb_sb)
            nc.vector.tensor_reduce(
                score[:, tcg : tcg + 1], vth, axis=mybir.AxisListType.X,
                op=ALU.add,
            )

        def batch_tail(bi, score):
            # batch tail: exp over [128, 16] with free-axis partial sums,
            # partition-sum via a tiny fp32 ones-matmul, then normalize.
            probs = small.tile([P, NTC], f32, tag="probs", name="probs")
            ssum = small.tile([P, 1], f32, tag="ssum", name="ssum")
            nc.scalar.activation(probs, score, AF.Exp, accum_out=ssum)
            # partition-sum via a bf16 ones-matmul; hi/lo split keeps the
            # normalizer at ~fp24 (a single bf16 cast would cost 0.4% rel)
            ssb = small.tile([P, 2], bf16, tag="ssb", name="ssb")
            nc.vector.tensor_copy(ssb[:, 0:1], ssum)
            nc.vector.tensor_sub(ssb[:, 1:2], ssum, ssb[:, 0:1])
            psS = ps_pool.tile([P, 2], f32, tag="psS", name="psS")
            nc.tensor.matmul(psS, ones_sb, ssb, start=True, stop=True)
            rs = small.tile([P, 1], f32, tag="rs", name="rs")
            sS = small.tile([P, 1], f32, tag="sS", name="sS")
            nc.vector.tensor_reduce(
                sS, psS, axis=mybir.AxisListType.X, op=ALU.add
            )
            nc.vector.reciprocal(rs, sS)
            probn = small.tile([P, NTC], f32, tag="probn", name="probn")
            nc.vector.tensor_scalar_mul(probn, probs, rs)
            nc.sync.dma_start(out[bi].rearrange("tc p -> p tc"), probn)

        first = True
        pending_tail = None
        for bi in [bi for _ in range(repeat) for bi in range(BPC)]:
            enct_b = enct[bi].rearrange("(ko p) t -> p ko t", p=P)
            score = small.tile([P, NTC], f32, tag="score")
            for ti in range(NTT):
                if ti == 1 and pending_tail is not None:
                    batch_tail(*pending_tail)
                    pending_tail = None
                if first:
                    et = et0
                else:
                    et = enc_pool.tile([P, KO, TT], bf16, tag="et0")
                    for ko in range(0, KO, 4):
                        nc.sync.dma_start(
                            et[:, ko : ko + 4, :],
                            enct_b[:, ko : ko + 4, ti * TT : (ti + 1) * TT],
                        )
                if first:
                    # kc-outer over t-chunks 0-2 so the PE starts as soon as
                    # the first w2/et chunks land; tc 3 runs kc-inner after.
                    first = False
                    pes = [
                        (
                            pe_pool.tile([P, TT], f32, tag="pe", name=f"peL{i}"),
                            pe_pool.tile([P, TT], f32, tag="pe", name=f"peR{i}"),
                        )
                        for i in range(3)
                    ]
                    for kc in range(KO):
                        for tq in range(3):
                            st = et[:, kc, tq * P : (tq + 1) * P]
                            peL, peR = pes[tq]
                            nc.tensor.matmul(
                                peL, st, w2_sb[:, kc, 0:HH],
                                start=(kc == 0), stop=(kc == KO - 1),
                            )
                            nc.tensor.matmul(
                                peR, st, w2_sb[:, kc, HH:H],
                                start=(kc == 0), stop=(kc == KO - 1),
                            )
                    for tq in range(3):
                        tc_epilogue(bi, tq, pes[tq][0], pes[tq][1], score)
                    tqs = [3]
                else:
                    tqs = range(TQ)
                for tq in tqs:
                    # serial accumulation groups: consecutive matmuls to the
                    # same PSUM bank sustain ~272 ns/mm; alternating banks
                    # per kc (to share the stationary) measures ~304 ns/mm.
                    peL = pe_pool.tile([P, TT], f32, tag="pe")
                    peR = pe_pool.tile([P, TT], f32, tag="pe")
                    for kc in range(KO):
                        nc.tensor.matmul(
                            peL, et[:, kc, tq * P : (tq + 1) * P],
                            w2_sb[:, kc, 0:HH],
                            start=(kc == 0), stop=(kc == KO - 1),
                        )
                    for kc in range(KO):
                        nc.tensor.matmul(
                            peR, et[:, kc, tq * P : (tq + 1) * P],
                            w2_sb[:, kc, HH:H],
                            start=(kc == 0), stop=(kc == KO - 1),
                        )
                    tc_epilogue(bi, ti * TQ + tq, peL, peR, score)
            pending_tail = (bi, score)
        batch_tail(*pending_tail)

    nc.compile()
    return nc


def kernel(hidden, encoder_outputs, W, b, v):
    global _nc_cache, _in_maps_cache
    from concourse.bass_utils import run_bass_kernel_spmd

    hidden = np.asarray(hidden, dtype=np.float32)
    enc = np.asarray(encoder_outputs, dtype=np.float32)
    W = np.asarray(W, dtype=np.float32)
    b = np.asarray(b, dtype=np.float32)
    v = np.asarray(v, dtype=np.float32)

    w2t = np.ascontiguousarray(W[:, H:].T).astype(_BF16)      # (2H, H) bf16
    u = hidden @ W[:, :H].T + b                               # (B, H) fp32
    vb = np.ascontiguousarray(
        np.broadcast_to(v.astype(_BF16)[None], (P, H))
    )
    ones = np.ones((P, P), dtype=_BF16)
    # (B, 2H, T) bf16 — contraction dim on partitions, t contiguous
    enct = np.ascontiguousarray(enc.transpose(0, 2, 1)).astype(_BF16)

    if _nc_cache is None:
        _nc_cache = _build_nc()
    nc = _nc_cache

    in_maps = []
    for c in range(NCORES):
        bs = c * BPC
        in_maps.append(
            {
                "enct": enct[bs : bs + BPC],
                "w2t": w2t,
                "ubc": np.ascontiguousarray(
                    np.broadcast_to(u[None, bs : bs + BPC], (P, BPC, H))
                ),
                "vbc": vb,
                "ones": ones,
            }
        )

    _in_maps_cache = in_maps
    res = run_bass_kernel_spmd(nc, in_maps, list(range(NCORES)))
    # device layout is [BPC, NTC, P]; t = tc*P + p
    outs = [res.results[c]["out"].reshape(BPC, T) for c in range(NCORES)]
    return np.concatenate(outs, axis=0)[:, None, :].astype(np.float32)



# revision 4
# speedup vs baseline: 1.0597x; 1.0331x over previous
"""Additive-attention scoring kernel for Trainium2 (Bass/Tile, 8 NeuronCores).

Computes softmax_t( v . tanh( W @ cat(hidden, enc)[b,t] + b ) ) for
hidden (B,H), enc (B,T,2H), W (H,3H), b (H,), v (H,)  ->  (B,1,T).

Restructured vs the classic layout: enc chunks are the STATIONARY operand
([k,t] 128x128) and W2 streams as the MOVING operand ([k,h] 128x512), so
PSUM tiles are [t, h].  Benefits:
  - the v-dot reduction is over the FREE axis -> fused mult+reduce on the
    (otherwise idle) Vector engine instead of 8 extra PE matmuls/t-tile
  - each stationary serves 2 moving matmuls (h halves) -> half the
    LDWEIGHTS traffic
  - scores land as [128t, 16] across partitions -> softmax tail is
    parallel instead of serial on one partition
  - u[b] = W1 @ hidden[b] + b is computed on the HOST (tiny GEMM) and
    shipped replicated across partitions; no W1/hidden on device

Sharding: data-parallel over batch, 2 batches per core.
"""

import numpy as np
import ml_dtypes

B, T, H = 16, 2048, 1024
K2 = 2 * H          # contraction dim of the big matmul
NCORES = 8
BPC = B // NCORES   # batches per core

P = 128
KO = K2 // P        # 16 k-chunks
TT = 512            # t-tile (et DMA granularity)
NTT = T // TT       # 4 et tiles per batch
TQ = TT // P        # 4 t-chunks per et tile
NTC = T // P        # 16 t-chunks per batch
HH = H // 2         # moving h-half (one PSUM bank of fp32)

_BF16 = ml_dtypes.bfloat16

_nc_cache = None
_in_maps_cache = None


def _build_nc(repeat=1):
    """Build the SPMD kernel. repeat>1 duplicates the compute body inside the
    NEFF (same inputs/outputs) — used only for differential device timing."""
    from contextlib import ExitStack

    import concourse.tile as tile
    from concourse import bacc, mybir

    f32 = mybir.dt.float32
    bf16 = mybir.dt.bfloat16
    AF = mybir.ActivationFunctionType
    ALU = mybir.AluOpType

    nc = bacc.Bacc()

    enct = nc.dram_tensor("enct", [BPC, K2, T], bf16, kind="ExternalInput")
    w2t = nc.dram_tensor("w2t", [K2, H], bf16, kind="ExternalInput")
    ubc = nc.dram_tensor("ubc", [P, BPC, H], f32, kind="ExternalInput")
    vbc = nc.dram_tensor("vbc", [P, H], bf16, kind="ExternalInput")
    ones = nc.dram_tensor("ones", [P, P], bf16, kind="ExternalInput")
    out = nc.dram_tensor("out", [BPC, NTC, P], f32, kind="ExternalOutput")

    with tile.TileContext(nc) as tc, ExitStack() as ctx:
        consts = ctx.enter_context(tc.tile_pool(name="consts", bufs=1))
        enc_pool = ctx.enter_context(tc.tile_pool(name="enc", bufs=4))
        th_pool = ctx.enter_context(tc.tile_pool(name="th", bufs=3))
        eu_pool = ctx.enter_context(tc.tile_pool(name="eu", bufs=2))
        vth_pool = ctx.enter_context(tc.tile_pool(name="vth", bufs=2))
        pe_pool = ctx.enter_context(tc.tile_pool(name="pe", bufs=6, space="PSUM"))
        ps_pool = ctx.enter_context(tc.tile_pool(name="ps", bufs=1, space="PSUM"))
        small = ctx.enter_context(tc.tile_pool(name="small", bufs=4))

        # Loads chunked in consumption order: the first matmuls need only
        # w2[kc=0] + et0[kc=0]; per-kc DMA (384 KB ~ 1.1 us) stays ahead of
        # the PE's 6 matmuls/kc (~1.3 us) in the first tile's kc-outer phase.
        w2_sb = consts.tile([P, KO, H], bf16)
        w2_r = w2t.rearrange("(ko p) h -> p ko h", p=P)
        ub_sb = consts.tile([P, BPC, H], f32)
        vb_sb = consts.tile([P, H], bf16)
        ones_sb = consts.tile([P, P], bf16)
        enct_b0 = enct[0].rearrange("(ko p) t -> p ko t", p=P)
        et0 = enc_pool.tile([P, KO, TT], bf16)
        for g in range(KO // 2):
            ko = 2 * g
            nc.sync.dma_start(w2_sb[:, ko : ko + 2, :], w2_r[:, ko : ko + 2, :])
            eng = nc.gpsimd if g < 2 else nc.sync
            eng.dma_start(et0[:, ko : ko + 2, :], enct_b0[:, ko : ko + 2, 0:TT])
            if g == 1:
                nc.gpsimd.dma_start(ub_sb, ubc[:])
                nc.gpsimd.dma_start(vb_sb, vbc[:])
                nc.gpsimd.dma_start(ones_sb, ones[:])

        def tc_epilogue(bi, tcg, peL, peR, score):
            # DVE adds u (reading PSUM, writing SBUF — DVE never writes
            # PSUM, PE-drain vs engine-access hazards stay one-directional),
            # ACT applies tanh, then one fused DVE mult+reduce forms the
            # t-chunk scores.
            eu = eu_pool.tile([P, 2 * TT], f32, tag="eu")
            th = th_pool.tile([P, 2 * TT], bf16, tag="th")
            nc.vector.tensor_add(eu[:, 0:TT], peL, ub_sb[:, bi, 0:HH])
            nc.scalar.activation(th[:, 0:TT], eu[:, 0:TT], AF.Tanh)
            nc.vector.tensor_add(eu[:, TT : 2 * TT], peR, ub_sb[:, bi, HH:H])
            nc.scalar.activation(th[:, TT : 2 * TT], eu[:, TT : 2 * TT], AF.Tanh)
            vth = vth_pool.tile([P, 2 * TT], bf16, tag="vth")
            nc.vector.tensor_mul(vth, th, vb_sb)
            nc.vector.tensor_reduce(
                score[:, tcg : tcg + 1], vth, axis=mybir.AxisListType.X,
                op=ALU.add,
            )

        def batch_tail(bi, score):
            # batch tail: exp over [128, 16] with free-axis partial sums,
            # partition-sum via a tiny fp32 ones-matmul, then normalize.
            probs = small.tile([P, NTC], f32, tag="probs", name="probs")
            ssum = small.tile([P, 1], f32, tag="ssum", name="ssum")
            nc.scalar.activation(probs, score, AF.Exp, accum_out=ssum)
            # partition-sum via a bf16 ones-matmul; hi/lo split keeps the
            # normalizer at ~fp24 (a single bf16 cast would cost 0.4% rel)
            ssb = small.tile([P, 2], bf16, tag="ssb", name="ssb")
            nc.vector.tensor_copy(ssb[:, 0:1], ssum)
            nc.vector.tensor_sub(ssb[:, 1:2], ssum, ssb[:, 0:1])
            psS = ps_pool.tile([P, 2], f32, tag="psS", name="psS")
            nc.tensor.matmul(psS, ones_sb, ssb, start=True, stop=True)
            rs = small.tile([P, 1], f32, tag="rs", name="rs")
            sS = small.tile([P, 1], f32, tag="sS", name="sS")
            nc.vector.tensor_reduce(
                sS, psS, axis=mybir.AxisListType.X, op=ALU.add
            )
            nc.vector.reciprocal(rs, sS)
            probn = small.tile([P, NTC], f32, tag="probn", name="probn")
            nc.vector.tensor_scalar_mul(probn, probs, rs)
            nc.sync.dma_start(out[bi].rearrange("tc p -> p tc"), probn)

        first = True
        pending_tail = None
        for bi in [bi for _ in range(repeat) for bi in range(BPC)]:
            enct_b = enct[bi].rearrange("(ko p) t -> p ko t", p=P)
            score = small.tile([P, NTC], f32, tag="score")
            for ti in range(NTT):
                if ti == 1 and pending_tail is not None:
                    batch_tail(*pending_tail)
                    pending_tail = None
                if first:
                    et = et0
                else:
                    et = enc_pool.tile([P, KO, TT], bf16, tag="et0")
                    qs = [nc.sync, nc.scalar, nc.gpsimd]
                    for qi, ko in enumerate(range(0, KO, 4)):
                        qs[(qi + ti + NTT * bi) % 3].dma_start(
                            et[:, ko : ko + 4, :],
                            enct_b[:, ko : ko + 4, ti * TT : (ti + 1) * TT],
                        )
                if first:
                    # kc-outer over t-chunks 0-2 so the PE starts as soon as
                    # the first w2/et chunks land; tc 3 runs kc-inner after.
                    first = False
                    pes = [
                        (
                            pe_pool.tile([P, TT], f32, tag="pe", name=f"peL{i}"),
                            pe_pool.tile([P, TT], f32, tag="pe", name=f"peR{i}"),
                        )
                        for i in range(3)
                    ]
                    for kc in range(KO):
                        for tq in range(3):
                            st = et[:, kc, tq * P : (tq + 1) * P]
                            peL, peR = pes[tq]
                            nc.tensor.matmul(
                                peL, st, w2_sb[:, kc, 0:HH],
                                start=(kc == 0), stop=(kc == KO - 1),
                            )
                            nc.tensor.matmul(
                                peR, st, w2_sb[:, kc, HH:H],
                                start=(kc == 0), stop=(kc == KO - 1),
                            )
                    for tq in range(3):
                        tc_epilogue(bi, tq, pes[tq][0], pes[tq][1], score)
                    tqs = [3]
                else:
                    tqs = range(TQ)
                for tq in tqs:
                    # serial accumulation groups: consecutive matmuls to the
                    # same PSUM bank sustain ~272 ns/mm; alternating banks
                    # per kc (to share the stationary) measures ~304 ns/mm.
                    peL = pe_pool.tile([P, TT], f32, tag="pe")
                    peR = pe_pool.tile([P, TT], f32, tag="pe")
                    for kc in range(KO):
                        nc.tensor.matmul(
                            peL, et[:, kc, tq * P : (tq + 1) * P],
                            w2_sb[:, kc, 0:HH],
                            start=(kc == 0), stop=(kc == KO - 1),
                        )
                    for kc in range(KO):
                        nc.tensor.matmul(
                            peR, et[:, kc, tq * P : (tq + 1) * P],
                            w2_sb[:, kc, HH:H],
                            start=(kc == 0), stop=(kc == KO - 1),
                        )
                    tc_epilogue(bi, ti * TQ + tq, peL, peR, score)
            pending_tail = (bi, score)
        batch_tail(*pending_tail)

    nc.compile()
    return nc


def kernel(hidden, encoder_outputs, W, b, v):
    global _nc_cache, _in_maps_cache
    from concourse.bass_utils import run_bass_kernel_spmd

    hidden = np.asarray(hidden, dtype=np.float32)
    enc = np.asarray(encoder_outputs, dtype=np.float32)
    W = np.asarray(W, dtype=np.float32)
    b = np.asarray(b, dtype=np.float32)
    v = np.asarray(v, dtype=np.float32)

    w2t = np.ascontiguousarray(W[:, H:].T).astype(_BF16)      # (2H, H) bf16
    u = hidden @ W[:, :H].T + b                               # (B, H) fp32
    vb = np.ascontiguousarray(
        np.broadcast_to(v.astype(_BF16)[None], (P, H))
    )
    ones = np.ones((P, P), dtype=_BF16)
    # (B, 2H, T) bf16 — contraction dim on partitions, t contiguous
    enct = np.ascontiguousarray(enc.transpose(0, 2, 1)).astype(_BF16)

    if _nc_cache is None:
        _nc_cache = _build_nc()
    nc = _nc_cache

    in_maps = []
    for c in range(NCORES):
        bs = c * BPC
        in_maps.append(
            {
                "enct": enct[bs : bs + BPC],
                "w2t": w2t,
                "ubc": np.ascontiguousarray(
                    np.broadcast_to(u[None, bs : bs + BPC], (P, BPC, H))
                ),
                "vbc": vb,
                "ones": ones,
            }
        )

    _in_maps_cache = in_maps
    res = run_bass_kernel_spmd(nc, in_maps, list(range(NCORES)))
    # device layout is [BPC, NTC, P]; t = tc*P + p
    outs = [res.results[c]["out"].reshape(BPC, T) for c in range(NCORES)]
    return np.concatenate(outs, axis=0)[:, None, :].astype(np.float32)

